# revision 8
# baseline (speedup 1.0000x reference)
"""Trainium2 Bass kernel: embedding-lookup -> mean-pool -> dot(weights).

out[b] = sum_l s[idx[b,l]],  s = embed_table @ (weights/L)   (V=100000, D=100)

Gather strategy (per core, 2048 batch rows, 409,600 tokens):
  - s striped 16-way across partitions: tab16[p, e] = s[16e + (p&15)],
    e < 6272 (25KB/partition, identical stripes in each 16-partition group).
  - 8x ap_gather (GPSIMD, all 8 Q7 cores in parallel): each 16-partition
    group g gathers its own token list L_g (its 256 batch rows x 200 tokens)
    by m = v>>4; output [128, 6400] holds, at partition p, s[16*m_i + (p&15)].
  - DVE: multiply by a host-shipped bf16 lane mask (j_i == p&15), reduce
    each row's 200-token run -> rs[128, 256] partial sums per partition.
  - PE: W8^T @ rs with W8[p, m] = (p>>4 == m) sums the 16 partitions of each
    group -> psum [8, 256] = all 2048 row outputs.

Vocab-parallel s precompute (12544 rows/core, strided row assignment so the
local s chunk is already stripe-ordered) + AllGather, as in the classic
data-parallel embedding recipe. Host does layout only: row re-ordering of the
table, index splitting (v>>4, v&15), wrap layouts, concat of outputs.
"""

import os
import sys

import numpy as np

for _p in ("/opt/trn_rl_repo",):
    if os.path.isdir(_p) and _p not in sys.path:
        sys.path.insert(0, _p)

from concourse import bacc, bass, mybir, tile  # noqa: E402
from concourse.bass_utils import run_bass_kernel_spmd  # noqa: E402

F32 = mybir.dt.float32
BF16 = mybir.dt.bfloat16
FP8 = mybir.dt.float8e4
I16 = mybir.dt.int16
P = 128
NCORES = 8

B, L, D, V = 16384, 200, 100, 100000
RPP = 98  # vocab rows per partition (per core): 128*98*8 = 100352 >= V
VPC = P * RPP  # 12544 vocab rows per core
NE = VPC * NCORES // 16  # 6272 stripe entries per partition
ROWS_PER_CORE = B // NCORES  # 2048
ROWS_PER_GROUP = ROWS_PER_CORE // 8  # 256
TOK_PER_GROUP = ROWS_PER_GROUP * L  # 51200
NI = 6400  # idxs per ap_gather per group (32 rows' runs)
NT = TOK_PER_GROUP // NI  # 8 gather instructions


def build_program(mask_dtype=FP8, gat_bufs=2):
    nc = bacc.Bacc(
        "TRN2", target_bir_lowering=False, debug=False, num_devices=NCORES
    )
    tab_t = nc.dram_tensor("tab", [P, RPP * D], F32, kind="ExternalInput")
    w_t = nc.dram_tensor("w", [P, D], F32, kind="ExternalInput")
    idx_t = nc.dram_tensor("idx", [P, TOK_PER_GROUP // 16], I16, kind="ExternalInput")
    msk_t = nc.dram_tensor("msk", [P, TOK_PER_GROUP], mask_dtype, kind="ExternalInput")
    w8_t = nc.dram_tensor("w8", [P, 8], F32, kind="ExternalInput")
    out_t = nc.dram_tensor("out", [8, ROWS_PER_GROUP], F32, kind="ExternalOutput")

    with tile.TileContext(nc) as tc:
        with tc.tile_pool(name="dr", bufs=1, space="DRAM") as dr:
            with tc.tile_pool(name="keep", bufs=1) as keep:
                # small hot-phase inputs first, on the (otherwise idle) sync
                # queue, so the first gather's deps land as early as possible
                idx_sb = keep.tile([P, TOK_PER_GROUP // 16], I16)
                nc.sync.dma_start(idx_sb[:], idx_t[:])
                w8_sb = keep.tile([P, 8], F32)
                nc.sync.dma_start(w8_sb[:], w8_t[:])
                rs = keep.tile([P, ROWS_PER_GROUP], F32)

                with tc.tile_pool(name="pre", bufs=1) as pre:
                    # ---- stage 1: local striped s chunk = (table slice) @ (w/L);
                    # big table loads on the scalar queue, DVE pipelined in 4
                    # chunks ----
                    tab_sb = pre.tile([P, RPP * D], F32)
                    w_sb = pre.tile([P, D], F32)
                    nc.scalar.dma_start(w_sb[:], w_t[:])
                    s_sb = pre.tile([P, RPP], F32)
                    NCH = 4
                    CH = RPP // NCH  # 24; last chunk takes the remainder
                    bounds = [(h * CH, (h + 1) * CH if h < NCH - 1 else RPP)
                              for h in range(NCH)]
                    for h, (r0, r1) in enumerate(bounds):
                        nc.scalar.dma_start(
                            tab_sb[:, r0 * D : r1 * D], tab_t[:, r0 * D : r1 * D]
                        )
                        prod_sb = pre.tile(
                            [P, (r1 - r0) * D], F32, tag="prod", name=f"pr{h}"
                        )
                        nc.vector.tensor_tensor(
                            out=prod_sb[:].rearrange("p (r d) -> p r d", d=D),
                            in0=tab_sb[:, r0 * D : r1 * D].rearrange(
                                "p (r d) -> p r d", d=D
                            ),
                            in1=w_sb[:].unsqueeze(1).to_broadcast([P, r1 - r0, D]),
                            op=mybir.AluOpType.mult,
                        )
                        nc.vector.tensor_reduce(
                            out=s_sb[:, r0:r1].unsqueeze(2),
                            in_=prod_sb[:].rearrange("p (r d) -> p r d", d=D),
                            axis=mybir.AxisListType.X,
                            op=mybir.AluOpType.add,
                        )

                    # ---- stage 2: AllGather striped s ----
                    s_part = dr.tile([P, RPP], F32)
                    nc.scalar.dma_start(s_part[:], s_sb[:])
                    s_full = dr.tile([NCORES * RPP, P], F32, addr_space="Shared")
                    nc.gpsimd.collective_compute(
                        "AllGather",
                        mybir.AluOpType.bypass,
                        replica_groups=[list(range(NCORES))],
                        ins=[s_part.opt()],
                        outs=[s_full.opt()],
                    )

                    # ---- stage 3: load striped table into SBUF ----
                    # flat s_full element index = c*12544 + (16*gam + j)*98 + k
                    # tab16[p, (c*8+gam)*98 + k] = s[16*e + j], j = p & 15
                    tab16 = keep.tile([P, NE], F32)
                    s_flat = s_full[:].rearrange("a b -> (a b)")
                    engines = [nc.sync, nc.scalar]
                    for g2 in range(8):
                        src = bass.AP(
                            s_flat.tensor,
                            0,
                            [[RPP, 16], [VPC, 8], [16 * RPP, 8], [1, RPP]],
                        )
                        engines[g2 % 2].dma_start(
                            tab16[16 * g2 : 16 * g2 + 16, :].rearrange(
                                "p (c g k) -> p c g k", g=8, k=RPP
                            ),
                            src,
                        )

                # ---- stage 4: gather + mask + run-reduce ----
                # fp8 masks (0/1 exact) are small enough to preload ALL tiles
                # before the gather phase starts: no DMA writes contend with
                # the Q7 gather reads/writes mid-phase. The final tile is
                # split in half to shorten the post-gather DVE tail.
                msk_all = keep.tile([P, NT * NI], mask_dtype)
                nc.sync.dma_start(msk_all[:], msk_t[:])
                subtiles = [(t * NI, NI) for t in range(NT - 1)]
                subtiles += [((NT - 1) * NI, NI // 2), ((NT - 1) * NI + NI // 2, NI // 2)]
                with tc.tile_pool(name="gat", bufs=gat_bufs) as gat:
                    for si, (o, n) in enumerate(subtiles):
                        gth = gat.tile([P, n], F32, tag="gth", name=f"g{si}")
                        nc.gpsimd.ap_gather(
                            gth[:],
                            tab16[:],
                            idx_sb[:, o // 16 : (o + n) // 16],
                            channels=P,
                            num_elems=NE,
                            d=1,
                            num_idxs=n,
                        )
                        msel = gat.tile([P, n], F32, tag="msel", name=f"s{si}")
                        nc.vector.tensor_tensor(
                            out=msel[:], in0=gth[:], in1=msk_all[:, o : o + n],
                            op=mybir.AluOpType.mult,
                        )
                        nc.vector.tensor_reduce(
                            out=rs[:, o // 200 : (o + n) // 200].unsqueeze(2),
                            in_=msel[:].rearrange("p (r l) -> p r l", l=200),
                            axis=mybir.AxisListType.X,
                            op=mybir.AluOpType.add,
                        )

                # ---- stage 5: PE group-sum over the 16 partitions of each group ----
                with (
                    tc.tile_pool(name="ps", bufs=1, space="PSUM") as ps,
                    tc.tile_pool(name="fin", bufs=1) as fin,
                ):
                    psum = ps.tile([8, ROWS_PER_GROUP], F32)
                    nc.tensor.matmul(psum[:], w8_sb[:], rs[:])
                    out_sb = fin.tile([8, ROWS_PER_GROUP], F32)
                    nc.any.tensor_copy(out_sb[:], psum[:])
                    nc.sync.dma_start(out_t[:], out_sb[:])
    nc.compile()
    return nc


def make_in_maps(word_idxs, embed_table, weights):
    idx = np.asarray(word_idxs).astype(np.int64)
    tab = np.asarray(embed_table, dtype=np.float32)
    w = np.asarray(weights, dtype=np.float32).reshape(-1)
    tab_pad = np.zeros((VPC * NCORES, D), dtype=np.float32)
    tab_pad[:V] = tab
    w_c = np.ascontiguousarray(
        np.broadcast_to((w / np.float32(L))[None, :], (P, D))
    ).astype(np.float32)
    w8 = np.zeros((P, 8), dtype=np.float32)
    w8[np.arange(P), np.arange(P) >> 4] = 1.0

    # striped vocab-row assignment: v(c, p, k) = c*VPC + 16*(98*(p>>4)+k) + (p&15)
    p_ar = np.arange(P)
    k_ar = np.arange(RPP)
    vmat = 16 * (RPP * (p_ar[:, None] >> 4) + k_ar[None, :]) + (p_ar[:, None] & 15)

    in_maps = []
    for c in range(NCORES):
        tab_c = np.ascontiguousarray(
            tab_pad[c * VPC + vmat].reshape(P, RPP * D)
        )
        rows = idx[c * ROWS_PER_CORE : (c + 1) * ROWS_PER_CORE]  # [2048, 200]
        Lg = rows.reshape(8, TOK_PER_GROUP)  # group g: rows g*256..., in order
        m = (Lg >> 4).astype(np.int16)  # [8, 51200]
        j = (Lg & 15).astype(np.int64)
        # idx wrap: idx_t[16g+w, t*400+s] = m[g, t*6400 + s*16 + w]
        mw = m.reshape(8, NT, NI // 16, 16)  # [g, t, s, w]
        idx_t = np.ascontiguousarray(
            mw.transpose(0, 3, 1, 2).reshape(P, NT * (NI // 16))
        )
        # mask: msk[16g+u, i] = (j[g, i] == u)
        u = np.arange(16)
        msk = (j[:, None, :] == u[None, :, None])  # [8, 16, 51200]
        import ml_dtypes

        msk_bf = np.ascontiguousarray(
            msk.reshape(P, TOK_PER_GROUP).astype(ml_dtypes.float8_e4m3)
        )
        in_maps.append(
            {"tab": tab_c, "w": w_c, "idx": idx_t, "msk": msk_bf, "w8": w8}
        )
    return in_maps


def unshard_out(results):
    parts = []
    for c in range(NCORES):
        o = np.asarray(results[c]["out"])  # [8, 256]: row c*2048 + g*256 + n
        parts.append(o.reshape(-1))
    return np.concatenate(parts).reshape(-1, 1).astype(np.float32)


_CACHED_NC = None


def _get_nc():
    global _CACHED_NC
    if _CACHED_NC is None:
        _CACHED_NC = build_program()
    return _CACHED_NC


def run(word_idxs, embed_table, weights, trace=False, **spmd_kwargs):
    nc = _get_nc()
    in_maps = make_in_maps(word_idxs, embed_table, weights)
    res = run_bass_kernel_spmd(
        nc, in_maps, core_ids=list(range(NCORES)), trace=trace, **spmd_kwargs
    )
    out = unshard_out(res.results)
    return out, res


def kernel(word_idxs, embed_table, weights):
    out, _ = run(word_idxs, embed_table, weights, trace=False)
    return out
